# revision 37
# baseline (speedup 1.0000x reference)
"""Trainium2 Bass kernel for nn_DifferentiableFSA.

The whole per-token network (embedding + Wn-proj + Wm-MLP + gelu + layernorm +
score head) is a pure function of the token id (vocab V=21), so it collapses
into tiny per-vocab tables computed once on device.  Per-token work is then: a
20-way table gather (compare+scale planes on DVE, summed exactly by the PE with
0/1 stationaries), softmax/cumsum structure over L done as matmuls with
block-triangular 0/1 matrices (fp32r, 1 cyc/row), and a short elementwise tail
done in a transposed "packed" (128, x) layout to keep DVE op costs low.

Layout per core (B_core = 1024 rows):
  partition p = c*64 + l   (c in {0,1} batch half, l in 0..63)
  free      n in 0..511    (batch row b = c*512 + n)
"""

import numpy as np
import ml_dtypes

import concourse.bass as bass
import concourse.bacc as bacc
import concourse.tile as tile
from concourse import mybir
from concourse.bass_utils import run_bass_kernel_spmd

F32 = mybir.dt.float32
F32R = mybir.dt.float32r
BF16 = mybir.dt.bfloat16

B, L, H, V = 8192, 64, 64, 21
NCORES = 8
BC = B // NCORES            # 1024 rows per core
NB = BC // 2                # 512 free width
P = 128
LN10 = float(np.log(10.0))
AX = mybir.AxisListType
OP = mybir.AluOpType
ACTF = mybir.ActivationFunctionType


# ----------------------------------------------------------------------------
# host-side constants (input independent)
# ----------------------------------------------------------------------------
def _build_consts():
    c = {}
    # numeric features per vocab id v: [digit_value, is_digit, op_type, is_operator]
    v = np.arange(V, dtype=np.float32)
    isdig = ((v >= 4) & (v <= 13)).astype(np.float32)
    dv = (v - 4.0) * isdig
    isop = ((v >= 14) & (v <= 17)).astype(np.float32)
    optype = np.where(v == 14, 1.0, 0.0) + np.where(v == 15, 2.0, 0.0) \
        + np.where(v == 16, 3.0, 0.0) + np.where(v == 17, 4.0, 0.0)
    nt = np.stack([dv, isdig, optype, isop], axis=-1)        # (21, 4)
    c["ntt"] = np.ascontiguousarray(nt.T).astype(np.float32)  # (4, 21)

    c["ones1"] = np.ones((1, P), dtype=np.float32)
    c["id128"] = np.eye(P, dtype=np.float32)
    c["idr"] = np.eye(P, dtype=np.float32)

    # block matrices over l within each c-half: out[m,n] = sum_k M[k,m] rhs[k,n]
    def blk(f):
        m = np.zeros((P, P), dtype=np.float32)
        k_ = np.arange(L)
        for cc in range(2):
            m[cc * L:(cc + 1) * L, cc * L:(cc + 1) * L] = f(
                k_[:, None], k_[None, :]).astype(np.float32)
        return m
    c["ltri"] = blk(lambda k, m: k <= m)   # inclusive cumsum
    c["stl"] = blk(lambda k, m: k < m)     # exclusive cumsum (cum_before)
    c["sut"] = blk(lambda k, m: k > m)     # total - inclusive cumsum

    # hist selectors: for vocab v, out row (c*32 + v) sums over l of c-half
    sel = np.zeros((P, 20, 64), dtype=np.float32)
    for i, vv in enumerate(range(1, V)):
        for cc in range(2):
            sel[cc * L:(cc + 1) * L, i, cc * 32 + vv] = 1.0
    c["sel"] = sel

    osel = np.zeros((P, 2), dtype=np.float32)     # contract l per c-half
    osel[0:L, 0] = 1.0
    osel[L:P, 1] = 1.0
    c["osel"] = osel
    c["csel"] = np.ascontiguousarray(osel.T)      # (2,128) broadcast c -> (c,l)
    csel8 = np.zeros((2, 8), dtype=np.float32)    # broadcast c -> rows (c*4+j)
    for j in range(4):
        for cc in range(2):
            csel8[cc, cc * 4 + j] = 1.0
    c["csel8"] = csel8
    return c


_CONSTS = _build_consts()


# ----------------------------------------------------------------------------
# per-core input maps
# ----------------------------------------------------------------------------
def _pack_layout():
    """column layout inside the two (128, W) f32 packs"""
    A, B = {}, {}
    off = [0]
    def put(d, name, rows, width):
        d[name] = (rows, off[0], width)
        off[0] += width
    put(A, "id128", P, P); put(A, "emb", V, H); put(A, "wn", 4, H)
    put(A, "bn", V, H); put(A, "wm", H, H); put(A, "bm", V, H)
    put(A, "gam", V, H); put(A, "bet", V, H); put(A, "wst", V, H)
    put(A, "wo", H, 4); put(A, "bo8", 8, 1); put(A, "ntt", 4, V)
    put(A, "ones1", 1, P); put(A, "onec", P, 1); put(A, "csel", 2, P)
    put(A, "csel8", 2, 8); put(A, "wrbrc", 2, P); put(A, "wrbva", 2, P)
    put(A, "brt", P, 1)
    wa = off[0]
    off[0] = 0
    put(B, "sel", P, 20 * 64)
    wb = off[0]
    off[0] = 0
    C = {}
    put(C, "ltri", P, P); put(C, "stl", P, P); put(C, "sut", P, P)
    put(C, "osel", P, 2); put(C, "csel", 2, P); put(C, "csel8", 2, 8)
    wc = off[0]
    return A, wa, B, wb, C, wc


_PLA, _WA, _PLB, _WB, _PLC, _WC = _pack_layout()


def _prep_inputs(token_ids, emb, Wn, bn, Wm, bm, gamma, beta, Ws, bs, Wo, bo,
                 Wr, br):
    token_ids = np.asarray(token_ids)
    f32 = lambda x: np.ascontiguousarray(np.asarray(x), dtype=np.float32)
    vals = {
        "id128": _CONSTS["id128"], "emb": f32(emb), "wn": f32(Wn),
        "bn": np.tile(f32(bn)[None, :], (V, 1)),
        "wm": f32(Wm), "bm": np.tile(f32(bm)[None, :], (V, 1)),
        "gam": np.tile(f32(gamma)[None, :], (V, 1)),
        "bet": np.tile(f32(beta)[None, :], (V, 1)),
        "wst": np.tile(f32(Ws).reshape(1, H), (V, 1)),
        "wo": f32(Wo), "ntt": _CONSTS["ntt"], "ones1": _CONSTS["ones1"],
        "onec": np.ones((P, 1), np.float32), "csel": _CONSTS["csel"],
        "csel8": _CONSTS["csel8"], "brt": np.tile(f32(br), 2).reshape(P, 1),
        "ltri": _CONSTS["ltri"], "stl": _CONSTS["stl"], "sut": _CONSTS["sut"],
        "sel": _CONSTS["sel"].reshape(P, 20 * 64), "osel": _CONSTS["osel"],
    }
    bo8 = np.zeros((8, 1), dtype=np.float32)      # rows (c*4+j)
    for j in range(4):
        for cc in range(2):
            bo8[cc * 4 + j, 0] = np.float32(np.asarray(bo).reshape(-1)[j])
    vals["bo8"] = bo8
    for q, nm in ((0, "wrbrc"), (1, "wrbva")):
        w2 = np.zeros((2, P), dtype=np.float32)
        for cc in range(2):
            w2[cc, cc * H:(cc + 1) * H] = f32(Wr)[q]
        vals[nm] = w2

    wpka = np.zeros((P, _WA), dtype=np.float32)
    for nm, (rows, o, w) in _PLA.items():
        wpka[0:rows, o:o + w] = vals[nm].reshape(rows, w)
    wpkb = np.zeros((P, _WB), dtype=np.float32)
    for nm, (rows, o, w) in _PLB.items():
        wpkb[0:rows, o:o + w] = vals[nm].reshape(rows, w)
    wpkc = np.zeros((P, _WC), dtype=ml_dtypes.bfloat16)
    for nm, (rows, o, w) in _PLC.items():
        wpkc[0:rows, o:o + w] = vals[nm].reshape(rows, w)

    in_maps = []
    for core in range(NCORES):
        tk = token_ids[core * BC:(core + 1) * BC]          # (1024, 64) int32
        tt = tk.reshape(2, NB, L).transpose(0, 2, 1).reshape(P, NB)
        m = {"wpka_d": wpka, "wpkb_d": wpkb, "wpkc_d": wpkc,
             "tok_d": np.ascontiguousarray(tt).astype(ml_dtypes.bfloat16)}
        in_maps.append(m)
    return in_maps


# ----------------------------------------------------------------------------
# the tile program
# ----------------------------------------------------------------------------
def _program(tc, outs, ins, stage=99):
    nc = tc.nc
    sb = tc.alloc_tile_pool(name="sb", bufs=1)
    pmA = tc.alloc_tile_pool(name="pmA", bufs=1, space="PSUM")
    pmB = tc.alloc_tile_pool(name="pmB", bufs=1, space="PSUM")
    pmC = tc.alloc_tile_pool(name="pmC", bufs=1, space="PSUM")
    pmT = tc.alloc_tile_pool(name="pmT", bufs=4, space="PSUM")

    wpka = sb.tile([P, _WA], F32, tag="wpka", name="wpka")
    nc.sync.dma_start(out=wpka, in_=ins["wpka_d"])
    wpkb = sb.tile([P, _WB], F32R, tag="wpkb", name="wpkb")
    nc.sync.dma_start(out=wpkb, in_=ins["wpkb_d"])
    wpkc = sb.tile([P, _WC], BF16, tag="wpkc", name="wpkc")
    nc.sync.dma_start(out=wpkc, in_=ins["wpkc_d"])
    tok = sb.tile([P, NB], BF16, tag="tok", name="tok")
    nc.sync.dma_start(out=tok, in_=ins["tok_d"])

    r32 = lambda ap: ap if ap.dtype == F32R else ap.bitcast(F32R)
    f32v = lambda ap: ap.bitcast(F32) if ap.dtype == F32R else ap

    def slA(name, raw=False):
        rows, o, w = _PLA[name]
        ap = wpka[0:rows, o:o + w]
        return ap if raw else f32v(ap)

    def slB(name, raw=False):
        rows, o, w = _PLB[name]
        ap = wpkb[0:rows, o:o + w]
        return ap if raw else f32v(ap)

    def slC(name):
        rows, o, w = _PLC[name]
        return wpkc[0:rows, o:o + w]

    def bpair(x, shape, nm):
        xh = sb.tile(shape, BF16, tag="bh" + nm, name="bh" + nm)
        nc.vector.tensor_copy(xh, x)
        xl = sb.tile(shape, BF16, tag="bl" + nm, name="bl" + nm)
        nc.vector.tensor_tensor(xl, x, xh, OP.subtract)
        return xh, xl

    def pair_mm(out, lhs_pair, rhs_pair, all4=True):
        lh, ll = lhs_pair
        rh, rl = rhs_pair
        terms = [(lh, rh), (lh, rl), (ll, rh)] + ([(ll, rl)] if all4 else [])
        for i, (a, b) in enumerate(terms):
            nc.tensor.matmul(out, a, b, start=(i == 0),
                             stop=(i == len(terms) - 1))

    id128 = slA("id128"); emb = slA("emb"); wn = slA("wn"); bn = slA("bn")
    wm = slA("wm"); bm = slA("bm"); gam = slA("gam"); bet = slA("bet")
    wst = slA("wst"); wo = slA("wo"); bo8 = slA("bo8"); ntt = slA("ntt")
    ones1 = slA("ones1")
    wrbrc = slA("wrbrc"); wrbva = slA("wrbva"); brt = slA("brt")
    ltri = slC("ltri"); stl = slC("stl"); sut = slC("sut")
    oselb = slC("osel"); cselb = slC("csel"); csel8b = slC("csel8")
    idr = sb.tile([P, P], F32R, tag="idr", name="idr")
    nc.vector.tensor_copy(idr, slA("id128"))
    selr = slB("sel", raw=True)
    sel3 = selr.rearrange("p (i m) -> p i m", i=20)

    # ---- table phase (tiny, partitions = vocab) ----
    ntth = sb.tile([4, V], BF16, tag="ntth", name="ntth")
    nc.vector.tensor_copy(ntth, ntt)          # NT entries are small ints: exact
    wnp = bpair(wn, [4, H], "wn")
    ntwn = pmT.tile([V, H], F32, tag="t", name="ntwn")
    nc.tensor.matmul(ntwn, ntth, wnp[0], start=True, stop=False)
    nc.tensor.matmul(ntwn, ntth, wnp[1], start=False, stop=True)
    t1 = sb.tile([V, H], F32, tag="t1", name="t1")
    nc.vector.scalar_tensor_tensor(t1, emb, 8.0, ntwn, OP.mult, OP.add)
    nc.vector.tensor_tensor(t1, t1, bn, OP.add)
    t1t_p = pmT.tile([H, V], F32, tag="t", name="t1t_p")
    nc.tensor.transpose(t1t_p, t1, id128[0:V, 0:V])
    t1t = sb.tile([H, V], F32, tag="t1t", name="t1t")
    nc.vector.tensor_copy(t1t, t1t_p)
    t1tp = bpair(t1t, [H, V], "t1t")
    wmp = bpair(wm, [H, H], "wm")
    t2p = pmT.tile([V, H], F32, tag="t", name="t2p")
    pair_mm(t2p, t1tp, wmp)
    t2pre = sb.tile([V, H], F32, tag="t2pre", name="t2pre")
    nc.vector.tensor_tensor(t2pre, t2p, bm, OP.add)
    er = sb.tile([V, H], F32, tag="er", name="er")
    nc.scalar.activation(er, t2pre, ACTF.Erf, scale=0.7071067811865476)
    t2 = sb.tile([V, H], F32, tag="t2", name="t2")
    nc.vector.scalar_tensor_tensor(t2, er, 1.0, t2pre, OP.add, OP.mult)
    nc.vector.tensor_scalar(t2, t2, 0.5, None, OP.mult)
    mu = sb.tile([V, 1], F32, tag="mu", name="mu")
    nc.vector.tensor_reduce(mu, t2, AX.X, OP.add)
    nc.vector.tensor_scalar(mu, mu, 1.0 / H, None, OP.mult)
    xc = sb.tile([V, H], F32, tag="xc", name="xc")
    nc.vector.tensor_scalar(xc, t2, mu, None, OP.subtract)
    sq = sb.tile([V, H], F32, tag="sq", name="sq")
    nc.vector.tensor_tensor(sq, xc, xc, OP.mult)
    var = sb.tile([V, 1], F32, tag="var", name="var")
    nc.vector.tensor_reduce(var, sq, AX.X, OP.add)
    nc.vector.tensor_scalar(var, var, 1.0 / H, 1e-6, OP.mult, OP.add)
    sd = sb.tile([V, 1], F32, tag="sd", name="sd")
    nc.scalar.activation(sd, var, ACTF.Sqrt)
    rstd = sb.tile([V, 1], F32, tag="rstd", name="rstd")
    nc.vector.reciprocal(rstd, sd)
    t3 = sb.tile([V, H], F32, tag="t3", name="t3")
    nc.vector.tensor_scalar(t3, xc, rstd, None, OP.mult)
    nc.vector.tensor_tensor(t3, t3, gam, OP.mult)
    nc.vector.tensor_tensor(t3, t3, bet, OP.add)
    t3t_p = pmT.tile([H, V], F32, tag="t", name="t3t_p")
    nc.tensor.transpose(t3t_p, t3, id128[0:V, 0:V])
    t3t = sb.tile([H, V], F32, tag="t3t", name="t3t")
    nc.vector.tensor_copy(t3t, t3t_p)
    # scores S = T3 @ Ws  (bs dropped: softmax shift invariant)
    junk = sb.tile([V, H], F32, tag="junk", name="junk")
    nc.vector.tensor_tensor(junk, t3, wst, OP.mult)
    S = sb.tile([V, 1], F32, tag="S", name="S")
    nc.vector.tensor_reduce(S, junk, AX.X, OP.add)
    srow_p = pmT.tile([1, V], F32, tag="t", name="srow_p")
    nc.tensor.transpose(srow_p, S, id128[0:V, 0:V])
    srow = sb.tile([1, V], F32, tag="srow", name="srow")
    nc.vector.tensor_copy(srow, srow_p)
    nc.vector.memset(srow[:, 0:1], -1e9)          # pad token -> exp = 0
    m1 = sb.tile([1, 1], F32, tag="m1", name="m1")
    nc.vector.tensor_reduce(m1, srow, AX.X, OP.max)
    negm1 = sb.tile([1, 1], F32, tag="negm1", name="negm1")
    nc.vector.tensor_scalar(negm1, m1, -1.0, None, OP.mult)
    erow = sb.tile([1, V], F32, tag="erow", name="erow")
    nc.scalar.activation(erow, srow, ACTF.Exp, bias=negm1)
    ones1b = sb.tile([1, P], BF16, tag="ones1b", name="ones1b")
    nc.vector.tensor_copy(ones1b, ones1)
    erowp = bpair(erow, [1, V], "erow")
    eb_p = pmT.tile([P, V], F32, tag="t", name="eb_p")
    nc.tensor.matmul(eb_p, ones1b, erowp[0], start=True, stop=False)
    nc.tensor.matmul(eb_p, ones1b, erowp[1], start=False, stop=True)
    e16f = sb.tile([P, V], F32, tag="e16f", name="e16f")
    nc.vector.tensor_copy(e16f, eb_p)
    # ST2 maps hist rows (c*32+v) -> (num cols c*4+j, denom cols 8+c)
    t3tp = bpair(t3t, [H, V], "t3t")
    wop = bpair(wo, [H, 4], "wo")
    st2p = pmT.tile([64, 10], F32, tag="t", name="st2p")
    for i, (a, b) in enumerate(((t3tp[0], wop[0]), (t3tp[0], wop[1]),
                                (t3tp[1], wop[0]), (t3tp[1], wop[1]))):
        nc.tensor.matmul(st2p[0:V, 0:4], a, b, start=(i == 0), stop=(i == 3),
                         skip_group_check=True)
    for i, (a, b) in enumerate(((t3tp[0], wop[0]), (t3tp[0], wop[1]),
                                (t3tp[1], wop[0]), (t3tp[1], wop[1]))):
        nc.tensor.matmul(st2p[32:32 + V, 4:8], a, b, start=(i == 0),
                         stop=(i == 3), skip_group_check=True)
    st2f = sb.tile([64, 10], F32, tag="st2", name="st2f")
    nc.vector.memset(st2f, 0.0)
    nc.vector.tensor_copy(st2f[0:V, 0:4], st2p[0:V, 0:4])
    nc.vector.tensor_copy(st2f[32:32 + V, 4:8], st2p[32:32 + V, 4:8])
    nc.vector.tensor_copy(st2f[0:V, 8:9], slA("onec")[0:V, :])
    nc.vector.tensor_copy(st2f[32:32 + V, 9:10], slA("onec")[32:32 + V, :])

    if stage <= 1:
        nc.sync.dma_start(out=outs["rw_d"][0:P, 0:V], in_=f32v(e16f))
        for p_ in (pmT, pmC, pmB, pmA, sb):
            p_.release()
        return
    # ---- gather planes + PE accumulations ----
    planes = sb.tile([P, 20, NB], F32R, tag="planes", name="planes")
    eacc = pmA.tile([P, NB], F32, tag="a", name="eacc")
    hist = pmB.tile([64, NB], F32, tag="b", name="hist")
    for i in range(20):
        nc.vector.tensor_scalar(
            planes[:, i, :], tok, float(i + 1), e16f[:, i + 1:i + 2],
            OP.is_equal, OP.mult)
    for i in range(20):
        nc.tensor.matmul(eacc, idr, planes[:, i, :],
                         start=(i == 0), stop=(i == 19))
    for i in range(20):
        nc.tensor.matmul(hist, sel3[:, i, :], planes[:, i, :],
                         start=(i == 0), stop=(i == 19))
    e = sb.tile([P, NB], F32, tag="e", name="e")
    nc.scalar.copy(e, eacc)
    h = sb.tile([64, NB], F32, tag="h", name="h")
    nc.vector.tensor_copy(h, hist)
    hp = bpair(h, [64, NB], "h")
    st2np = bpair(st2f[:, 0:8], [64, 8], "st2n")
    num_p = pmC.tile([8, NB], F32, tag="c", name="num_p")
    pair_mm(num_p, st2np, hp)
    numr = sb.tile([8, NB], F32, tag="numr", name="numr")
    nc.vector.tensor_copy(numr, num_p)
    st2d = sb.tile([64, 2], BF16, tag="st2d", name="st2d")
    nc.vector.tensor_copy(st2d, st2f[:, 8:10])   # 0/1: exact
    den_p = pmC.tile([2, NB], F32, tag="c", name="den_p")
    nc.tensor.matmul(den_p, st2d, hp[0], start=True, stop=False)
    nc.tensor.matmul(den_p, st2d, hp[1], start=False, stop=True)
    denr = sb.tile([2, NB], F32, tag="denr", name="denr")
    nc.vector.tensor_copy(denr, den_p)

    if stage <= 2:
        for p_ in (pmT, pmC, pmB, pmA, sb):
            p_.release()
        return
    # ---- softmax normalize: reciprocal in packed (128, 8) layout ----
    denP_p = pmT.tile([P, 8], F32, tag="t", name="denP_p")
    for k in range(4):
        nc.tensor.transpose(denP_p[:, k * 2:(k + 1) * 2],
                            denr[:, k * P:(k + 1) * P], id128[0:2, 0:2])
    denP = sb.tile([P, 4, 2], F32, tag="denP", name="denP")
    nc.vector.tensor_copy(denP, denP_p)
    rdP = sb.tile([P, 4, 2], F32, tag="rdP", name="rdP")
    nc.vector.reciprocal(rdP, denP)
    rdrow_p = pmT.tile([2, NB], F32, tag="t", name="rdrow_p")
    for k in range(4):
        nc.tensor.transpose(rdrow_p[:, k * P:(k + 1) * P],
                            rdP[:, k, :], id128)
    rd = sb.tile([2, NB], F32, tag="rd", name="rd")
    nc.vector.tensor_copy(rd, rdrow_p)
    rdp = bpair(rd, [2, NB], "rd")
    rdb = pmC.tile([P, NB], F32, tag="c", name="rdb")
    nc.tensor.matmul(rdb, cselb, rdp[0], start=True, stop=False)
    nc.tensor.matmul(rdb, cselb, rdp[1], start=False, stop=True)
    rw = sb.tile([P, NB], F32, tag="rw", name="rw")
    nc.vector.tensor_tensor(rw, e, rdb, OP.mult)
    rwp = bpair(rw, [P, NB], "rw")

    if stage <= 3:
        for p_ in (pmT, pmC, pmB, pmA, sb):
            p_.release()
        return
    # ---- cumsums and digit masks ----
    cum = pmA.tile([P, NB], F32, tag="a", name="cum")
    nc.tensor.matmul(cum, ltri, rwp[0], start=True, stop=False)
    nc.tensor.matmul(cum, ltri, rwp[1], start=False, stop=True)
    cumb = pmB.tile([P, NB], F32, tag="b", name="cumb")
    nc.tensor.matmul(cumb, stl, rwp[0], start=True, stop=False)
    nc.tensor.matmul(cumb, stl, rwp[1], start=False, stop=True)
    i1 = sb.tile([P, NB], BF16, tag="i1", name="i1")
    nc.gpsimd.tensor_scalar(i1, tok, 3.5, None, OP.is_ge)
    idg = sb.tile([P, NB], BF16, tag="idg", name="idg")
    nc.vector.scalar_tensor_tensor(idg, tok, 13.5, i1, OP.is_le, OP.mult)
    dvm = sb.tile([P, NB], BF16, tag="dvm", name="dvm")
    nc.vector.scalar_tensor_tensor(dvm, tok, 4.0, idg, OP.subtract, OP.mult)
    nidg = sb.tile([P, NB], BF16, tag="nidg", name="nidg")
    nc.gpsimd.tensor_scalar(nidg, idg, -1.0, None, OP.mult)
    lm = sb.tile([P, NB], F32, tag="lm", name="lm")
    nc.vector.scalar_tensor_tensor(lm, cum, 1.0, nidg, OP.subtract, OP.mult)
    rm = sb.tile([P, NB], F32, tag="rm", name="rm")
    nc.vector.tensor_tensor(rm, cumb, idg, OP.mult)

    # ---- soft assemble (both sides) ----
    qs = {}
    for side, msk in (("l", lm), ("r", rm)):
        pool, ptag = (pmA, "a") if side == "l" else (pmB, "b")
        mp = bpair(msk, [P, NB], "m" + side)
        tl = pool.tile([P, NB], F32, tag=ptag, name="tl" + side)
        nc.tensor.matmul(tl, sut, mp[0], start=True, stop=False)
        nc.tensor.matmul(tl, sut, mp[1], start=False, stop=True)
        px = sb.tile([P, NB], F32, tag="px" + side, name="px" + side)
        nc.vector.tensor_tensor(px, tl, msk, OP.mult)
        pw = sb.tile([P, NB], F32, tag="pw" + side, name="pw" + side)
        nc.scalar.activation(pw, px, ACTF.Exp, scale=LN10)
        g = sb.tile([P, NB], F32, tag="g" + side, name="g" + side)
        nc.gpsimd.tensor_tensor(g, msk, dvm, OP.mult)
        q = sb.tile([P, NB], F32, tag="q" + side, name="q" + side)
        nc.vector.tensor_tensor(q, pw, g, OP.mult)
        qs[side] = bpair(q, [P, NB], "q" + side)
    lv_p = pmT.tile([2, NB], F32, tag="t", name="lv_p")
    nc.tensor.matmul(lv_p, oselb, qs["l"][0], start=True, stop=False)
    nc.tensor.matmul(lv_p, oselb, qs["l"][1], start=False, stop=True)
    rv_p = pmT.tile([2, NB], F32, tag="t", name="rv_p")
    nc.tensor.matmul(rv_p, oselb, qs["r"][0], start=True, stop=False)
    nc.tensor.matmul(rv_p, oselb, qs["r"][1], start=False, stop=True)
    lvr = sb.tile([2, NB], F32, tag="lvr", name="lvr")
    nc.scalar.copy(lvr, lv_p)
    rvr = sb.tile([2, NB], F32, tag="rvr", name="rvr")
    nc.scalar.copy(rvr, rv_p)
    half_out = lambda t: bass.AP(tensor=t.tensor, offset=t.offset,
                                 ap=[[NB, 2], [1, NB]])
    nc.sync.dma_start(out=half_out(outs["left_d"]), in_=lvr)
    nc.sync.dma_start(out=half_out(outs["right_d"]), in_=rvr)

    if stage <= 4:
        for p_ in (pmT, pmC, pmB, pmA, sb):
            p_.release()
        return
    # ---- op logits (rows c*4+j) ----
    rd8 = pmC.tile([8, NB], F32, tag="c", name="rd8")
    nc.tensor.matmul(rd8, csel8b, rdp[0], start=True, stop=False)
    nc.tensor.matmul(rd8, csel8b, rdp[1], start=False, stop=True)
    ol = sb.tile([8, NB], F32, tag="ol", name="ol")
    nc.vector.tensor_tensor(ol, numr, rd8, OP.mult)
    nc.vector.tensor_scalar(ol, ol, bo8, None, OP.add)

    # ---- pack to (128, x) land: chunk transposes ----
    def pack(rowt, nrow, nm):
        dst = pmT.tile([P, 4 * nrow], F32, tag="t", name="pk_" + nm)
        for k in range(4):
            nc.tensor.transpose(dst[:, k * nrow:(k + 1) * nrow],
                                rowt[:, k * P:(k + 1) * P],
                                id128[0:nrow, 0:nrow])
        s = sb.tile([P, 4, nrow], F32, tag="pks_" + nm, name="pks_" + nm)
        nc.vector.tensor_copy(s, dst)
        return s

    lvP = pack(lvr, 2, "lv")          # (128, 4k, 2c)
    rvP = pack(rvr, 2, "rv")
    olP = pack(ol, 8, "ol")           # (128, 4k, (c*4+j))
    for cc in range(2):
        nc.sync.dma_start(
            out=bass.AP(tensor=outs["oplog_d"].tensor, offset=cc * NB * 4,
                        ap=[[4, P], [NB * 4 // 4 * 4, 0]][:1] + [[P * 4, 4], [1, 4]]),
            in_=olP[:, :, cc * 4:(cc + 1) * 4])

    lv8 = lvP.rearrange("p k c -> p (k c)")
    rv8 = rvP.rearrange("p k c -> p (k c)")
    absr = sb.tile([P, 8], F32, tag="absr", name="absr")
    nc.vector.tensor_scalar(absr.bitcast(mybir.dt.uint32), rv8.bitcast(mybir.dt.uint32),
                            0x7FFFFFFF, None, OP.bitwise_and)
    divok = sb.tile([P, 8], F32, tag="divok", name="divok")
    nc.vector.tensor_scalar(divok, absr, 1e-6, None, OP.is_gt)
    invok = sb.tile([P, 8], F32, tag="invok", name="invok")
    nc.vector.tensor_scalar(invok, absr, 1e-6, None, OP.is_le)
    sr = sb.tile([P, 8], F32, tag="sr", name="sr")
    nc.vector.tensor_tensor(sr, rv8, divok, OP.mult)
    nc.vector.tensor_tensor(sr, sr, invok, OP.add)
    rsr = sb.tile([P, 8], F32, tag="rsr", name="rsr")
    nc.vector.reciprocal(rsr, sr)

    res = sb.tile([P, 4, 8], F32, tag="res", name="res")   # (p, j, (k c))
    nc.vector.tensor_tensor(res[:, 0, :], lv8, rv8, OP.add)
    nc.vector.tensor_tensor(res[:, 1, :], lv8, rv8, OP.subtract)
    nc.vector.tensor_tensor(res[:, 2, :], lv8, rv8, OP.mult)
    nc.vector.tensor_tensor(res[:, 3, :], lv8, rsr, OP.mult)
    nc.vector.tensor_tensor(res[:, 3, :], res[:, 3, :], divok, OP.mult)
    val = sb.tile([P, 4, 8], F32, tag="val", name="val")
    nc.vector.memset(val[:, 0:3, :], 1.0)
    nc.vector.tensor_copy(val[:, 3, :], divok)

    eol = sb.tile([P, 4, 8], F32, tag="eol", name="eol")   # (p, j, (k c))
    nc.scalar.activation(eol, olP.rearrange("p k (c j) -> p j (k c)", c=2),
                         ACTF.Exp)
    wres = sb.tile([P, 4, 8], F32, tag="wres", name="wres")
    nc.vector.tensor_tensor(wres, eol, res, OP.mult)
    wval = sb.tile([P, 4, 8], F32, tag="wval", name="wval")
    nc.vector.tensor_tensor(wval, eol, val, OP.mult)
    sres = sb.tile([P, 8], F32, tag="sres", name="sres")
    nc.vector.tensor_reduce(sres, wres.rearrange("p j f -> p f j"), AX.X, OP.add)
    sval = sb.tile([P, 8], F32, tag="sval", name="sval")
    nc.vector.tensor_reduce(sval, wval.rearrange("p j f -> p f j"), AX.X, OP.add)
    seol = sb.tile([P, 8], F32, tag="seol", name="seol")
    nc.vector.tensor_reduce(seol, eol.rearrange("p j f -> p f j"), AX.X, OP.add)
    rsum = sb.tile([P, 8], F32, tag="rsum", name="rsum")
    nc.vector.reciprocal(rsum, seol)
    resP = sb.tile([P, 8], F32, tag="resP", name="resP")
    nc.vector.tensor_tensor(resP, sres, rsum, OP.mult)
    valP = sb.tile([P, 8], F32, tag="valP", name="valP")
    nc.vector.tensor_tensor(valP, sval, rsum, OP.mult)

    # rc = sign(result) * log1p(|result|)
    absR = sb.tile([P, 8], F32, tag="absR", name="absR")
    nc.vector.tensor_scalar(absR.bitcast(mybir.dt.uint32), resP.bitcast(mybir.dt.uint32),
                            0x7FFFFFFF, None, OP.bitwise_and)
    # log1p(a) with range split: for a >= 2^62 use ln(a*2^-64) + 64 ln2
    big = sb.tile([P, 8], F32, tag="big", name="big")
    nc.vector.tensor_scalar(big, absR, float(2.0 ** 62), None, OP.is_ge)
    nb_ = sb.tile([P, 8], F32, tag="nb_", name="nb_")
    nc.vector.tensor_scalar(nb_, big, -1.0, 1.0, OP.mult, OP.add)
    small = sb.tile([P, 8], F32, tag="small", name="small")
    nc.vector.tensor_scalar(small, absR, float(2.0 ** -64), None, OP.mult)
    xsb = sb.tile([P, 8], F32, tag="xsb", name="xsb")
    nc.vector.tensor_tensor(xsb, small, big, OP.mult)
    xsn = sb.tile([P, 8], F32, tag="xsn", name="xsn")
    nc.vector.tensor_tensor(xsn, absR, nb_, OP.mult)
    lnin = sb.tile([P, 8], F32, tag="lnin", name="lnin")
    nc.vector.tensor_tensor(lnin, xsb, xsn, OP.add)
    nc.vector.tensor_tensor(lnin, lnin, nb_, OP.add)
    l1p = sb.tile([P, 8], F32, tag="l1p", name="l1p")
    nc.scalar.activation(l1p, lnin, ACTF.Ln)
    nc.vector.scalar_tensor_tensor(l1p, big, float(64.0 * np.log(2.0)), l1p,
                                   OP.mult, OP.add)
    sg = sb.tile([P, 8], F32, tag="sg", name="sg")
    nc.vector.tensor_scalar(sg, resP, 0.0, None, OP.is_gt)
    sg2 = sb.tile([P, 8], F32, tag="sg2", name="sg2")
    nc.vector.tensor_scalar(sg2, resP, 0.0, None, OP.is_lt)
    nc.vector.tensor_tensor(sg, sg, sg2, OP.subtract)
    rcP = sb.tile([P, 8], F32, tag="rcP", name="rcP")
    nc.vector.tensor_tensor(rcP, sg, l1p, OP.mult)

    # result / valid outputs straight from packed land: free index (k, c)


    if stage <= 5:
        for p_ in (pmT, pmC, pmB, pmA, sb):
            p_.release()
        return
    # ---- unpack rc/valid/result to rows, result embedding ----
    rows = {}
    for qi, (nm, src) in enumerate((("rc", rcP), ("va", valP), ("rs", resP))):
        row_p = pmT.tile([2, NB], F32, tag="t", name=f"unpk{qi}")
        for k in range(4):
            nc.tensor.transpose(
                row_p[:, k * P:(k + 1) * P],
                src.rearrange("p (k c) -> p k c", k=4)[:, k, :],
                id128)
        rows[nm] = sb.tile([2, NB], F32, tag="row" + nm, name="row" + nm)
        nc.vector.tensor_copy(rows[nm], row_p)
    half_out2 = lambda t: bass.AP(tensor=t.tensor, offset=0,
                                  ap=[[NB, 2], [1, NB]])
    nc.sync.dma_start(out=half_out2(outs["result_d"]), in_=rows["rs"])
    nc.sync.dma_start(out=half_out2(outs["valid_d"]), in_=rows["va"])
    re_p = pmC.tile([P, NB], F32, tag="c", name="re_p")
    nc.tensor.matmul(re_p, wrbrc, rows["rc"], start=True, stop=False)
    nc.tensor.matmul(re_p, wrbva, rows["va"], start=False, stop=True)
    re = sb.tile([P, NB], F32, tag="re", name="re")
    nc.vector.tensor_scalar(re, re_p, brt, None, OP.add)

    # ---- transposed big outputs: rw and result_embedding ----
    for name, src in (("rw_d", rw), ("remb_d", re)):
        for cc in range(2):
            for k in range(4):
                chunk = pmT.tile([P, H], F32, tag="t", name=f"ot_{name}{cc}{k}")
                nc.tensor.transpose(
                    chunk, f32v(src[cc * H:(cc + 1) * H, k * P:(k + 1) * P]),
                    id128[cc * H:(cc + 1) * H, cc * H:(cc + 1) * H])
                stage = sb.tile([P, H], F32, tag="out_s", name=f"os_{name}{cc}{k}",
                                bufs=2)
                nc.vector.tensor_copy(stage, chunk)
                nc.sync.dma_start(
                    out=outs[name][cc * NB + k * P: cc * NB + (k + 1) * P, :],
                    in_=stage)

    for p_ in (pmT, pmC, pmB, pmA, sb):
        p_.release()


# ----------------------------------------------------------------------------
# build + run
# ----------------------------------------------------------------------------
_BUILT = {}


def _in_specs():
    return {
        "wpka_d": ((P, _WA), np.float32),
        "wpkb_d": ((P, _WB), np.float32),
        "wpkc_d": ((P, _WC), ml_dtypes.bfloat16),
        "tok_d": ((P, NB), ml_dtypes.bfloat16),
    }


def build_program(stage=99):
    if ("nc", stage) in _BUILT:
        return _BUILT[("nc", stage)]
    nc = bacc.Bacc("TRN2", target_bir_lowering=False, debug=False,
                   enable_asserts=False, num_devices=1)
    ins = {}
    for name, (shape, dtype) in _in_specs().items():
        dt_ = F32R if name == "wpkb_d" else mybir.dt.from_np(np.dtype(dtype))
        ins[name] = nc.dram_tensor(
            name, list(shape), dt_, kind="ExternalInput").ap()
    outs = {}
    for name, shape in (
            ("result_d", (BC, 1)), ("valid_d", (BC, 1)), ("remb_d", (BC, H)),
            ("left_d", (BC, 1)), ("right_d", (BC, 1)), ("oplog_d", (BC, 4)),
            ("rw_d", (BC, L))):
        outs[name] = nc.dram_tensor(name, list(shape), F32,
                                    kind="ExternalOutput").ap()
    with tile.TileContext(nc) as tc:
        _program(tc, outs, ins, stage=stage)
    nc.finalize()
    _BUILT[("nc", stage)] = nc
    return nc


def kernel(token_ids, emb, Wn, bn, Wm, bm, gamma, beta, Ws, bs, Wo, bo, Wr, br):
    nc = build_program()
    in_maps = _prep_inputs(token_ids, emb, Wn, bn, Wm, bm, gamma, beta,
                           Ws, bs, Wo, bo, Wr, br)
    r = run_bass_kernel_spmd(nc, in_maps, core_ids=list(range(NCORES)))
    res = r.results
    cat = lambda n: np.concatenate([res[c][n] for c in range(NCORES)], axis=0)
    return (cat("result_d"), cat("valid_d"), cat("remb_d"), cat("left_d"),
            cat("right_d"), cat("oplog_d"), cat("rw_d"))
